# revision 24
# baseline (speedup 1.0000x reference)
"""Trainium2 Bass kernel for the DSCBlockLBP dense-CNN block.

Reference computation (per full batch):
    BatchNorm2d (training-mode batch stats over (N,H,W)) -> depthwise 3x3
    conv (C=256 -> NW=512, multiplier 2, weights in {-1,0,1}) -> ReLU ->
    1x1 conv (512 -> 256) + bias -> residual add x.

Strategy (8 NeuronCores, data-parallel over batch N=8 -> 1 sample/core):
  * x is shipped host-side as a zero-padded bf16 [2, 128, 130, 130] image
    (halves HBM load traffic vs f32; bf16 matmul = f32r speed on TRN2 PE
    and rounding error ~0.1% rms, well under the 2e-2 gate).
  * KEY RESTRUCTURE vs the v1 kernel: the depthwise conv runs on RAW x.
    Because every depthwise output channel o depends on exactly one input
    channel c(o), BN folds into the post-matmul step:
        y_o = ReLU(s_{c(o)} * DW_o(x) + t_{c(o)} * wsum_o)
    evaluated as one ScalarE activation (scale/bias are per-partition APs)
    or a 2-op DVE sequence.  The depthwise matmul stream therefore starts
    as soon as the first x rows land and the whole load + bn_stats +
    AllReduce chain hides behind ~123us of PE work instead of
    serializing in front of it (~105us of the v1 kernel's 318us).
  * Depthwise: 9 accumulating bf16 matmuls per 128-out-channel block
    (K=64), block pairs on disjoint PE row-groups (partitions 0-63/64-127).
  * Pre-ReLU depthwise outputs spill PSUM->SBUF as bf16 into a ring
    (ScalarE/VectorE copies); the ReLU + 1x1 consumer lane starts once the
    AllReduced stats arrive (~60us) and interleaves with remaining
    depthwise strips on the PE.
  * Batch stats via DVE bn_stats/bn_aggr per 4-row group as DMA chunks
    land; cross-core reduction is one 2KB AllReduce of (mean_i, E[x^2]_i).
  * 1x1 conv: dense K=512 bf16 matmul; residual + conv bias fused in one
    DVE scalar_tensor_tensor from PSUM (in1 = resident bf16 x).
"""

import numpy as np

B, C, H, W = 8, 256, 128, 128
NW = 512
EPS = 1e-5
NCORES = 8
HP, WP = H + 2, W + 2  # padded
TH = 4                 # strip height (N per matmul = TH*W = 512)
NSTRIPS = H // TH
TAPS = [(dh, dw) for dh in (-1, 0, 1) for dw in (-1, 0, 1)]
LEAD = 18              # depthwise strips emitted before the first 1x1
SPILL_BUFS = 20        # ring depth (strips) for pre-ReLU spill tiles
# x DMA row chunks over the padded 130 rows: 7+11 rows first (so strip 0
# can start as early as possible), then 7x16
CHUNK_ROWS = [(0, 7), (7, 18)] + \
    [(18 + 16 * k, 34 + 16 * k) for k in range(7)]

_cached = {}


def _build_nc(single_core=False, repeat=1, taps_only=False,
              repeat_all=False):
    from contextlib import ExitStack

    import concourse.bass as bass
    import concourse.tile as tile
    from concourse import mybir

    f32 = mybir.dt.float32
    bf16 = mybir.dt.bfloat16
    AF = mybir.ActivationFunctionType
    ALU = mybir.AluOpType

    nc = bass.Bass("TRN2", target_bir_lowering=False, debug=False,
                   num_devices=1 if single_core else NCORES)

    x_d = nc.dram_tensor("x", [2, 128, HP, WP], bf16,
                         kind="ExternalInput").ap()
    taps_d = nc.dram_tensor("taps", [2, 128, 9, 128], bf16,
                            kind="ExternalInput").ap()
    w1t_d = nc.dram_tensor("w1t", [4, 128, 256], bf16,
                           kind="ExternalInput").ap()
    wsum_d = nc.dram_tensor("wsum", [4, 128], f32, kind="ExternalInput").ap()
    gamma_d = nc.dram_tensor("gamma2", [2, 128], f32,
                             kind="ExternalInput").ap()
    beta_d = nc.dram_tensor("beta2", [2, 128], f32, kind="ExternalInput").ap()
    b1_d = nc.dram_tensor("b12", [2, 128], f32, kind="ExternalInput").ap()
    out_d = nc.dram_tensor("out", [C, H, W], f32, kind="ExternalOutput").ap()

    with tile.TileContext(nc) as tc, ExitStack() as ctx:
        resident = ctx.enter_context(tc.tile_pool(name="resident", bufs=1))
        small = ctx.enter_context(tc.tile_pool(name="small", bufs=1))
        dram = ctx.enter_context(
            tc.tile_pool(name="dram", bufs=1, space="DRAM"))
        psy_pool = ctx.enter_context(
            tc.tile_pool(name="psy", bufs=4, space="PSUM"))
        psz_pool = ctx.enter_context(
            tc.tile_pool(name="psz", bufs=4, space="PSUM"))
        spill_pool = ctx.enter_context(
            tc.tile_pool(name="spill",
                         bufs=4 if taps_only else SPILL_BUFS))
        y_pool = ctx.enter_context(tc.tile_pool(name="ypool", bufs=3))
        z_pool = ctx.enter_context(tc.tile_pool(name="zpool", bufs=4))

        outer_reps = repeat if repeat_all else 1
        inner_reps = 1 if repeat_all else repeat
        for _rep in range(outer_reps):
          # -------- ACT spline-table warmup (sqrt set also holds relu/copy)
          warm = small.tile([128, 1], f32, name="warm")
          nc.vector.memset(warm, 1.0)
          nc.scalar.activation(warm, warm, AF.Sqrt)

          # -------- weights / params (scalar HWDGE ring; x chunks go on
          # the sync ring so the two issue streams run in parallel) ------
          taps_sb = []
          for p in range(2):
              tp = resident.tile([128, 9, 128], bf16, name=f"taps{p}")
              nc.scalar.dma_start(out=tp, in_=taps_d[p])
              taps_sb.append(tp)

          # -------- x load (pre-padded on host) + per-chunk bn_stats ---
          # bn_stats runs on flat contiguous 512-element windows of the
          # padded image (33 windows cover elements [0, 16896); the 4
          # leftover elements are pad zeros).  Pad zeros contribute
          # nothing to sum / sum-of-squares, so true stats are the
          # aggregate rescaled by 16896/16384.
          NGRP = 33
          xres = [resident.tile([128, HP, WP], bf16, name=f"xres{p}")
                  for p in range(2)]
          cstats = [small.tile([128, NGRP, 6], f32, name=f"cstats{p}")
                    for p in range(2)]
          g_done = 0
          for (r0, r1) in CHUNK_ROWS:
              for p in range(2):
                  nc.sync.dma_start(out=xres[p][:, r0:r1, :],
                                    in_=x_d[p, :, r0:r1, :])
              g_end = min(NGRP, (r1 * WP) // 512)
              for p in range(2):
                  base = xres[p]
                  for g in range(g_done, g_end):
                      flat = bass.AP(tensor=base.tensor,
                                     offset=base.offset + 512 * g,
                                     ap=[base.ap[0], [1, 512]])
                      nc.vector.bn_stats(cstats[p][:, g, :], flat)
              g_done = g_end

          w1t_sb = []
          for kb in range(4):
              wt = resident.tile([128, 256], bf16, name=f"w1t{kb}")
              nc.scalar.dma_start(out=wt, in_=w1t_d[kb])
              w1t_sb.append(wt)

          eps_sb = small.tile([128, 1], f32, name="eps_sb")
          nc.vector.memset(eps_sb, EPS)
          b1_sb = []
          for mb in range(2):
              bb = small.tile([128, 1], f32, name=f"b1_{mb}")
              nc.scalar.dma_start(out=bb, in_=b1_d[mb])
              b1_sb.append(bb)
          wsum_sb, gam_dup, bet_dup = [], [], []
          for j in range(4):
              ws = small.tile([128, 1], f32, name=f"ws{j}")
              nc.scalar.dma_start(out=ws, in_=wsum_d[j])
              wsum_sb.append(ws)
              # gamma/beta duplicated into out-block-j layout:
              # partition q <- channel 64j + q//2
              for (src_d, dst_list, nm) in ((gamma_d, gam_dup, "gd"),
                                            (beta_d, bet_dup, "bd")):
                  src = src_d[j // 2][64 * (j % 2):64 * (j % 2) + 64]
                  dup = bass.AP(tensor=src.tensor, offset=src.offset,
                                ap=[src.ap[0], [0, 2]])
                  t_ = small.tile([128, 1], f32, name=f"{nm}{j}")
                  nc.scalar.dma_start(out=t_, in_=dup)
                  dst_list.append(t_)

          # -------- local stats -> AllReduce --------------------------
          RATIO = (512.0 * NGRP) / (H * W)  # window count incl pad zeros
          stats_local = dram.tile([C, 2], f32, name="stats_local")
          for p in range(2):
              mv = small.tile([128, 2], f32, name=f"mv{p}")
              nc.vector.bn_aggr(mv, cstats[p])
              sl = small.tile([128, 2], f32, name=f"sl{p}")
              e2 = small.tile([128, 1], f32, name=f"e2{p}")
              nc.vector.tensor_mul(e2, mv[:, 0:1], mv[:, 0:1])
              nc.vector.tensor_add(e2, mv[:, 1:2], e2)
              nc.vector.tensor_scalar_mul(sl[:, 1:2], e2, RATIO)
              nc.vector.tensor_scalar_mul(sl[:, 0:1], mv[:, 0:1], RATIO)
              nc.sync.dma_start(out=stats_local[128 * p:128 * (p + 1), :],
                                in_=sl)

          stats_sum = dram.tile([C, 2], f32, name="stats_sum",
                                addr_space="Shared")
          if single_core:
              nc.gpsimd.dma_start(out=stats_sum, in_=stats_local)
          else:
              nc.gpsimd.collective_compute(
                  "AllReduce",
                  ALU.add,
                  replica_groups=[list(range(NCORES))],
                  ins=[stats_local.opt()],
                  outs=[stats_sum.opt()],
              )

          # -------- per-out-block scale/bias in dup layout ------------
          # (emitted mid-way through the depthwise strips -- see below --
          # so a late AllReduce doesn't stall ACT/DVE in front of the
          # spill copies the PE pipeline depends on)
          s_dup, bias_sb = [], []

          def emit_derivations():
              for j in range(4):
                  src = stats_sum[64 * j:64 * j + 64, :]
                  dup = bass.AP(tensor=src.tensor, offset=src.offset,
                                ap=[src.ap[0], [0, 2], src.ap[1]])
                  g = small.tile([128, 2], f32, name=f"g{j}")
                  nc.sync.dma_start(out=g, in_=dup)
                  mg = small.tile([128, 1], f32, name=f"mg{j}")
                  nc.vector.tensor_scalar_mul(mg, g[:, 0:1], 1.0 / NCORES)
                  var = small.tile([128, 1], f32, name=f"var{j}")
                  nc.vector.tensor_mul(var, mg, mg)
                  nc.vector.scalar_tensor_tensor(
                      out=var, in0=g[:, 1:2], scalar=1.0 / NCORES, in1=var,
                      op0=ALU.mult, op1=ALU.subtract)
                  sd = small.tile([128, 1], f32, name=f"sd{j}")
                  nc.scalar.activation(sd, var, AF.Sqrt, bias=eps_sb,
                                       scale=1.0)
                  rstd = small.tile([128, 1], f32, name=f"rstd{j}")
                  nc.vector.reciprocal(rstd, sd)
                  s_ = small.tile([128, 1], f32, name=f"s{j}")
                  nc.vector.tensor_mul(s_, gam_dup[j], rstd)
                  s_dup.append(s_)
                  t_ = small.tile([128, 1], f32, name=f"t{j}")
                  nc.vector.tensor_mul(t_, mg, s_)
                  nc.vector.tensor_sub(t_, bet_dup[j], t_)
                  bj = small.tile([128, 1], f32, name=f"bias{j}")
                  nc.vector.tensor_mul(bj, wsum_sb[j], t_)
                  bias_sb.append(bj)

          # -------- strip pipeline ------------------------------------
          spills = [[None] * 4 for _ in range(NSTRIPS)]

          def emit_dw(st):
              h0 = TH * st
              for p in range(2):
                  ps = [psy_pool.tile([128, TH, W], f32, name="psy")
                        for _ in range(2)]
                  for t, (dh, dw) in enumerate(TAPS):
                      # one full-array [128,128] weight load serves both
                      # row-group matmuls (ldweights=False keeps walrus
                      # from re-loading per matmul)
                      nc.tensor.ldweights(taps_sb[p][:, t, :])
                      for jj in range(2):
                          lo = 64 * jj
                          rhs = xres[p][lo:lo + 64,
                                        1 + h0 + dh:1 + h0 + dh + TH,
                                        1 + dw:1 + dw + W]
                          lhsT = taps_sb[p][lo:lo + 64, t, :]
                          mm = nc.tensor.matmul(ps[jj], lhsT, rhs,
                                                start=(t == 0), stop=(t == 8))
                          mm.ins.ldweights = False
                  for jj in range(2):
                      j = 2 * p + jj
                      sp = spill_pool.tile([128, TH, W], bf16, name=f"sp{j}")
                      # all spills on ACT: DVE is busy with bn_stats early
                      # and with ReLU/residual later
                      nc.scalar.copy(sp, ps[jj])
                      spills[st][j] = sp

          def emit_relu(st):
              y_sb = []
              for j in range(4):
                  yj = y_pool.tile([128, TH, W], bf16, name=f"y{j}")
                  sp = spills[st][j]
                  if j < 2:
                      nc.scalar.activation(yj, sp, AF.Relu,
                                           bias=bias_sb[j], scale=s_dup[j])
                  else:
                      nc.vector.tensor_scalar(yj, sp, s_dup[j], bias_sb[j],
                                              op0=ALU.mult, op1=ALU.add)
                      nc.vector.tensor_scalar_max(yj, yj, 0.0)
                  y_sb.append(yj)
              return y_sb

          def emit_consume2(sa, sb_):
              # paired strips: each 1x1 weight block is loaded once and
              # shared by both strips' matmuls
              ys = {sa: emit_relu(sa), sb_: emit_relu(sb_)}
              for mb in range(2):
                  pzs = {}
                  for st in (sa, sb_):
                      pzs[st] = psz_pool.tile([128, TH, W], f32, name="psz")
                  for kb in range(4):
                      for st in (sa, sb_):
                          mm = nc.tensor.matmul(
                              pzs[st], w1t_sb[kb][:, 128 * mb:128 * (mb + 1)],
                              ys[st][kb], start=(kb == 0), stop=(kb == 3))
                          mm.ins.ldweights = False
                  for st in (sa, sb_):
                      h0 = TH * st
                      zt = z_pool.tile([128, TH, W], f32, name="zt")
                      nc.vector.scalar_tensor_tensor(
                          out=zt, in0=pzs[st], scalar=b1_sb[mb],
                          in1=xres[mb][:, 1 + h0:1 + h0 + TH, 1:1 + W],
                          op0=ALU.add, op1=ALU.add)
                      nc.sync.dma_start(
                          out=out_d[128 * mb:128 * (mb + 1), h0:h0 + TH, :],
                          in_=zt)

          for _irep in range(inner_reps):
              if taps_only:
                  for st in range(NSTRIPS):
                      emit_dw(st)
                  continue
              for st in range(LEAD):
                  emit_dw(st)
                  if st == 12 and not s_dup:
                      emit_derivations()
              if not s_dup:
                  emit_derivations()
              # catch-up interleave: one consume strip-pair per remaining
              # dw strip so the post-depthwise (DVE-bound) tail stays short
              nxt = 0
              for i in range(NSTRIPS - LEAD):
                  if nxt + 1 <= LEAD + i - 1:
                      emit_consume2(nxt, nxt + 1)
                      nxt += 2
                  emit_dw(LEAD + i)
              while nxt < NSTRIPS:
                  emit_consume2(nxt, nxt + 1)
                  nxt += 2

    from drainfix_embedded import split_excess_waits, dedupe_ldweights
    dedupe_ldweights(nc)
    split_excess_waits(nc)
    return nc


# --- embedded drain fix (kernel.py must be self-contained) ----------------
import sys as _sys
import types as _types

_dfix = _types.ModuleType("drainfix_embedded")
_dfix_code = '''
from concourse import mybir


def split_excess_waits(nc, max_waits=1):
    """walrus (CoreV2/V3 CTRL lowering) accepts at most one sync-wait per
    instruction; Tile's tail drain can carry one wait per logical proc.
    Move the excess onto same-engine NOPs inserted just before."""
    for fn in nc.m.functions:
        for bb in fn.blocks:
            insts = bb.instructions
            i = 0
            while i < len(insts):
                ins = insts[i]
                si = ins.sync_info
                if si is not None and si.on_wait and len(si.on_wait) > max_waits:
                    waits = list(si.on_wait)
                    extra, keep = waits[:-max_waits], waits[-max_waits:]
                    ins.sync_info = mybir.SyncInfo(
                        on_wait=keep, on_update=list(si.on_update))
                    new_nops = []
                    for j in range(0, len(extra), max_waits):
                        nop = nc.sync.nop().ins
                        nop.engine = ins.engine
                        nop.sync_info = mybir.SyncInfo(
                            on_wait=extra[j:j + max_waits], on_update=[])
                        new_nops.append(nop)
                    last_bb = nc.m.functions[-1].blocks[-1]
                    for nop in new_nops:
                        if nop in last_bb.instructions:
                            last_bb.instructions.remove(nop)
                    for k, nop in enumerate(new_nops):
                        insts.insert(i + k, nop)
                    i += len(new_nops)
                i += 1


def dedupe_ldweights(nc):
    """Delete InstLdweights whose weights are already in the PE array:
    identical reloads (paired 1x1 matmuls) and half-row loads covered by a
    preceding explicit full 128-row load (depthwise row-group pairs).
    Sync waits/updates of a deleted load merge into the next PE
    instruction (its matmul), which is also marked non-self-loading."""
    PE = mybir.EngineType.PE

    def sig(ins):
        ap = ins.ins[0]
        return (str(ap.memref), str(ap.dtype), ap.offset,
                tuple(tuple(d) for d in ap.ap))

    def covered(loaded, s):
        if loaded is None or loaded[0] != s[0] or loaded[1] != s[1]:
            return False
        if loaded[2:] == s[2:]:
            return True
        la, sa = loaded[3], s[3]
        if (len(la) == 2 and len(sa) == 2 and la[1] == sa[1]
                and la[0][0] == sa[0][0] and la[0][1] == 128
                and sa[0][1] == 64
                and s[2] - loaded[2] in (0, 64 * la[0][0])):
            return True
        return False

    for fn in nc.m.functions:
        for bb in fn.blocks:
            insts = bb.instructions
            loaded = None
            i = 0
            while i < len(insts):
                ins = insts[i]
                if getattr(ins, "engine", None) != PE:
                    i += 1
                    continue
                if isinstance(ins, mybir.InstLdweights):
                    s = sig(ins)
                    if covered(loaded, s):
                        # find the following PE instruction (its matmul)
                        k = i + 1
                        while k < len(insts) and (
                                getattr(insts[k], "engine", None) != PE):
                            k += 1
                        assert k < len(insts) and isinstance(
                            insts[k], mybir.InstMatmult), (
                            "deleted ldweights not followed by a PE matmul")
                        mm = insts[k]
                        si = ins.sync_info
                        if si is not None and (si.on_wait or si.on_update):
                            msi = mm.sync_info
                            mw = list(msi.on_wait) if msi else []
                            mu = list(msi.on_update) if msi else []
                            mm.sync_info = mybir.SyncInfo(
                                on_wait=list(si.on_wait) + mw,
                                on_update=list(si.on_update) + mu)
                        mm.ldweights = False
                        del insts[i]
                        continue
                    loaded = s
                elif isinstance(ins, mybir.InstMatmult):
                    pass
                else:
                    # any other PE op (nop etc.) leaves array state intact
                    pass
                i += 1
'''
exec(_dfix_code, _dfix.__dict__)
_sys.modules["drainfix_embedded"] = _dfix


def _np_bf16():
    from concourse import mybir
    return mybir.dt.np(mybir.dt.bfloat16)


def _host_prep(gamma, beta, lbp_w, w1, b1):
    bf16 = _np_bf16()
    lbp = np.ascontiguousarray(lbp_w, dtype=np.float32).reshape(NW, 9)
    taps = np.zeros((2, 128, 9, 128), np.float32)
    q = np.arange(128)
    cl = q % 64
    for p in range(2):
        j = 2 * p + (q // 64)          # out-block per partition row
        o0 = 128 * j + 2 * cl          # first of the two out-channels
        for jj in range(2):
            taps[p, q, :, 2 * cl + jj] = lbp[o0 + jj, :]
    w1t = np.ascontiguousarray(
        w1.reshape(C, NW).T.reshape(4, 128, C), dtype=np.float32)
    wsum = lbp.sum(1).reshape(4, 128).astype(np.float32)
    return {
        "taps": taps.astype(bf16),
        "w1t": w1t.astype(bf16),
        "wsum": wsum,
        "gamma2": np.ascontiguousarray(gamma, np.float32).reshape(2, 128),
        "beta2": np.ascontiguousarray(beta, np.float32).reshape(2, 128),
        "b12": np.ascontiguousarray(b1, np.float32).reshape(2, 128),
    }


def _prep_x(x):
    """f32 [B, C, H, W] -> per-core pre-padded bf16 [B, 2, 128, HP, WP]."""
    bf16 = _np_bf16()
    xp = np.zeros((B, 2, 128, HP, WP), bf16)
    xb = np.ascontiguousarray(x, np.float32).reshape(B, 2, 128, H, W)
    xp[:, :, :, 1:1 + H, 1:1 + W] = xb.astype(bf16)
    return xp


def _run(x, gamma, beta, lbp_w, w1, b1, trace=False):
    from concourse.bass_utils import run_bass_kernel_spmd

    if "nc" not in _cached:
        _cached["nc"] = _build_nc()
    nc = _cached["nc"]

    shared = _host_prep(gamma, beta, lbp_w, w1, b1)
    xp = _prep_x(x)
    in_maps = [dict(shared, x=xp[i]) for i in range(NCORES)]
    res = run_bass_kernel_spmd(nc, in_maps, core_ids=list(range(NCORES)),
                               trace=trace)
    out = np.stack([res.results[i]["out"] for i in range(NCORES)], axis=0)
    return out.astype(np.float32), res


def kernel(x, gamma, beta, lbp_w, w1, b1):
    out, _ = _run(x, gamma, beta, lbp_w, w1, b1)
    return out
